# revision 1
# baseline (speedup 1.0000x reference)
"""GATv2 message-passing kernel for 8 Trainium2 NeuronCores (Bass/Tile).

Strategy
--------
Nodes are partitioned into 8 contiguous ranges (one per core). Every edge is
assigned to the core that owns its *receiver*, so each core computes the
complete softmax + weighted aggregation for its own nodes with no collectives.

Host-side preprocessing (index-driven data movement only, no FLOPs):
  * group edges by (core, receiver-tile-of-128), pad each tile's edge list to
    a common chunk count, and lay the edge features out transposed
    ([feat, edge]) for direct use as matmul operands;
  * pre-gather raw sender node features per edge (nodes[senders]) so the
    device reads a sequential stream instead of doing a random-access gather;
  * all weights/constants are pre-rounded to fp32r (fp32 with 11-bit
    mantissa) so the PE runs matmuls at full rate; products of fp32r inputs
    accumulate exactly in fp32.

Device pipeline per receiver tile (128 nodes), per group of <=4 edge chunks
(chunk = 128 edges), with zT meaning "transposed [dim, edge] layout":
  zT   = We.T@edgesT + Ws.T@sent_rawT + hr_tile@onehot   (PSUM accumulate)
  x    = PRelu(zT, 0.01)                                  (ACT)
  lgT  = ablk.T @ x                                       (logits [8, W])
  exT  = Exp(lgT)                                         (ACT)
  ex   = exT.T per chunk (tiny K=8 matmul)
  sprj = sent_raw @ Ws per chunk ([edge, dim] layout)
  msg  = sprj * broadcast(ex)                             (DVE)
  acc += onehot_en.T @ [msg | ex]                         (scatter matmul)
Epilogue per tile: out = U / (D + eps), DMA to the output rows.
"""
import os
import sys

sys.path.insert(0, "/opt/trn_rl_repo")

import numpy as np
import concourse.bass as bass
import concourse.bacc as bacc
import concourse.mybir as mybir
import concourse.tile as tile
from concourse.bass_utils import run_bass_kernel_spmd

F32 = mybir.dt.float32
F32R = mybir.dt.float32r

NCORES = 8
P = 128
HEADS = 8
HDIM = 16

LAST_EXEC_NS = None
LAST_PROFILE = None
LAST_BENCH_NS = None


def _round_f32r(a: np.ndarray) -> np.ndarray:
    u = np.ascontiguousarray(a, dtype=np.float32).view(np.uint32)
    r = (u + np.uint32(0x800)) & np.uint32(0xFFFFF000)
    return r.view(np.float32)


def kernel(nodes, edges, senders, receivers, Ws_k, Ws_b, Wr_k, Wr_b, We_k, We_b, a):
    global LAST_EXEC_NS, LAST_PROFILE

    nodes = np.asarray(nodes, dtype=np.float32)
    edges = np.asarray(edges, dtype=np.float32)
    senders = np.asarray(senders, dtype=np.int32)
    receivers = np.asarray(receivers, dtype=np.int32)
    Ws_k = np.asarray(Ws_k, dtype=np.float32)
    Ws_b = np.asarray(Ws_b, dtype=np.float32)
    Wr_k = np.asarray(Wr_k, dtype=np.float32)
    Wr_b = np.asarray(Wr_b, dtype=np.float32)
    We_k = np.asarray(We_k, dtype=np.float32)
    We_b = np.asarray(We_b, dtype=np.float32)
    a = np.asarray(a, dtype=np.float32)

    N, D = nodes.shape
    E = edges.shape[0]
    assert D == 128 and Ws_k.shape == (128, 128)
    assert N % NCORES == 0
    NLOC = N // NCORES
    NTILES = (NLOC + P - 1) // P
    NLOC_PAD = NTILES * P

    # ---------------- host-side sharding / layout ----------------
    core = receivers // NLOC
    rloc_in_core = receivers - core * NLOC
    tl = rloc_in_core // P                      # receiver tile within core
    recv_local = (rloc_in_core - tl * P).astype(np.float32)
    gt = core * NTILES + tl                     # global (core,tile) bucket

    order = np.argsort(gt, kind="stable")
    gt_sorted = gt[order]
    cnt = np.bincount(gt_sorted, minlength=NCORES * NTILES)
    T_max = max(1, int(-(-cnt.max() // P)))     # chunks per tile, all cores
    NCHUNK = NTILES * T_max
    E_pad = NCHUNK * P

    # slot of each (sorted) edge inside its core's stream
    starts = np.zeros(NCORES * NTILES + 1, dtype=np.int64)
    np.cumsum(cnt, out=starts[1:])
    rank = np.arange(E, dtype=np.int64) - starts[gt_sorted]
    slot = (gt_sorted % NTILES) * (T_max * P) + rank

    edges_f32r = _round_f32r(edges)
    sent_raw_f32r = _round_f32r(nodes)[senders]     # [E, 128] host gather

    EDG = np.zeros((NCORES, P, E_pad), dtype=np.float32)
    SRT = np.zeros((NCORES, P, E_pad), dtype=np.float32)
    RROW = np.full((NCORES, 1, E_pad), -1.0, dtype=np.float32)
    RLOC = np.full((NCORES, P, NCHUNK), -1.0, dtype=np.float32)
    for ci in range(NCORES):
        sel = order[gt_sorted // NTILES == ci]
        sl = slot[gt_sorted // NTILES == ci]
        EDG[ci][:, sl] = edges_f32r[sel].T
        SRT[ci][:, sl] = sent_raw_f32r[sel].T
        RROW[ci][0, sl] = recv_local[sel]
        RLOC[ci][sl % P, sl // P] = recv_local[sel]

    # local (per-core) transposed node features for the hr projection
    nodes_f32r = _round_f32r(nodes)
    NLT = np.zeros((NCORES, P, NLOC_PAD), dtype=np.float32)
    for ci in range(NCORES):
        NLT[ci][:, :NLOC] = nodes_f32r[ci * NLOC:(ci + 1) * NLOC].T

    # block-diagonal attention vector [128, 8]
    ablk = np.zeros((P, HEADS), dtype=np.float32)
    for h in range(HEADS):
        ablk[h * HDIM:(h + 1) * HDIM, h] = a[h]

    b_all = Ws_b + Wr_b + We_b
    add_bias = bool(np.any(b_all != 0.0))

    # const block layout (cols), all fp32r:
    #   0:128    We
    #   128:256  Ws           (128:384 doubles as the [Ws | junk] sentproj rhs)
    #   256:512  [Wr | zeros] (prologue rhs, padded to 256 for f32r full rate)
    #   512:520  ablk
    #   520:648  iota rows    (iota[p, j] = j)
    #   648:649  iotaC        (iotaC[p] = p)
    #   649:657  identity8    (I8 in partitions 0:8)
    #   657:785  ones         (all 1.0; row 0 used as [1,128] lhsT)
    #   785:1041 bias row     ([b_all | zeros] in row 0)
    CW = 1041
    CONST = np.zeros((P, CW), dtype=np.float32)
    CONST[:, 0:128] = We_k
    CONST[:, 128:256] = Ws_k
    CONST[:, 256:384] = Wr_k
    CONST[:, 512:520] = ablk
    CONST[:, 520:648] = np.arange(P, dtype=np.float32)[None, :]
    CONST[:, 648] = np.arange(P, dtype=np.float32)
    CONST[0:8, 649:657] = np.eye(8, dtype=np.float32)
    CONST[:, 657:785] = 1.0
    CONST[0, 785:913] = b_all
    CONST = _round_f32r(CONST)

    # ---------------- build the bass program ----------------
    GROUPS = []
    g0 = 0
    while g0 < T_max:
        GROUPS.append((g0, min(4, T_max - g0)))
        g0 += 4

    nc = bacc.Bacc("TRN2", target_bir_lowering=False, debug=False)

    d_edg = nc.declare_dram_parameter("EDG", [P, E_pad], F32R, isOutput=False)
    d_srt = nc.declare_dram_parameter("SRT", [P, E_pad], F32R, isOutput=False)
    d_rrow = nc.declare_dram_parameter("RROW", [1, E_pad], F32R, isOutput=False)
    d_rloc = nc.declare_dram_parameter("RLOC", [P, NCHUNK], F32, isOutput=False)
    d_nlt = nc.declare_dram_parameter("NLT", [P, NLOC_PAD], F32R, isOutput=False)
    d_cb = nc.declare_dram_parameter("CONST", [P, CW], F32R, isOutput=False)
    d_zer = nc.declare_dram_parameter("ZER", [P, 1024], F32R, isOutput=False)
    d_out = nc.declare_dram_parameter("OUT", [NLOC_PAD, P], F32, isOutput=True)

    PRELU = mybir.ActivationFunctionType.Prelu
    EXP = mybir.ActivationFunctionType.Exp
    COPY = mybir.ActivationFunctionType.Copy
    EQ = mybir.AluOpType.is_equal
    MUL = mybir.AluOpType.mult
    ADD = mybir.AluOpType.add

    with tile.TileContext(nc) as tc:
        with (
            tc.tile_pool(name="cst", bufs=1) as cpool,
            tc.tile_pool(name="sb", bufs=2) as sb,
            tc.tile_pool(name="ps1", bufs=2, space="PSUM") as ps1,
            tc.tile_pool(name="ps2", bufs=2, space="PSUM") as ps2,
            tc.tile_pool(name="ps_spj", bufs=1, space="PSUM") as ps_spj,
        ):
            cb = cpool.tile([P, CW], F32R)
            nc.sync.dma_start(out=cb[:], in_=d_cb[:])
            rloc = cpool.tile([P, NCHUNK], F32)
            nc.sync.dma_start(out=rloc[:], in_=d_rloc[:])
            nlt = cpool.tile([P, NLOC_PAD], F32R)
            nc.sync.dma_start(out=nlt[:], in_=d_nlt[:])
            hr_sb = cpool.tile([P, NLOC_PAD], F32R)


            c_We = cb[:, 0:128]
            c_Ws = cb[:, 128:256]
            c_WsPad = cb[:, 128:384]
            c_WrPad = cb[:, 256:512]
            c_ablk = cb[:, 512:520]
            c_iota = cb[:, 520:648].bitcast(F32)
            c_iotaC = cb[:, 648:649].bitcast(F32)
            c_id8 = cb[0:8, 649:657]
            c_ones = cb[0:1, 657:785]
            c_brow = cb[0:1, 785:1041]

            # ---- prologue: hr projection for local nodes ----
            for t in range(NTILES):
                pp = ps1.tile([P, 256], F32, tag="aux")
                nc.tensor.matmul(
                    out=pp[:], lhsT=nlt[:, t * P:(t + 1) * P], rhs=c_WrPad,
                    start=True, stop=not add_bias,
                )
                if add_bias:
                    nc.tensor.matmul(
                        out=pp[:], lhsT=cb[0:1, 657:658], rhs=c_brow,
                        start=False, stop=True,
                    )
                dst = hr_sb[:, t * P:(t + 1) * P]
                if t % 2 == 0:
                    nc.scalar.activation(dst, pp[:, 0:128], COPY)
                else:
                    nc.vector.tensor_copy(out=dst, in_=pp[:, 0:128])

            # ---- main loop over receiver tiles ----
            for t in range(NTILES):
                co = t * T_max * P
                edg = sb.tile([P, T_max * P], F32R, tag="edg")
                nc.sync.dma_start(out=edg[:], in_=d_edg[:, co:co + T_max * P])
                srt = sb.tile([P, T_max * P], F32R, tag="srt")
                nc.sync.dma_start(out=srt[:], in_=d_srt[:, co:co + T_max * P])
                rr = sb.tile([1, T_max * P], F32R, tag="rr")
                nc.sync.dma_start(out=rr[:], in_=d_rrow[:, co:co + T_max * P])

                acc = ps2.tile([P, 256], F32, tag="acc")
                hr_t = hr_sb[:, t * P:(t + 1) * P]
                n_sc = 0

                for gi, (gc0, ncg) in enumerate(GROUPS):
                    W = ncg * P
                    csl = slice(gc0 * P, gc0 * P + W)

                    bc = ps1.tile([P, W], F32, tag="aux")
                    nc.tensor.matmul(out=bc[:], lhsT=c_ones, rhs=rr[0:1, csl],
                                     start=True, stop=True)
                    ohne = sb.tile([P, W], F32R, tag="ohne")
                    nc.vector.tensor_scalar(out=ohne[:], in0=bc[:],
                                            scalar1=c_iotaC, scalar2=None, op0=EQ)

                    zT = ps2.tile([P, W], F32, tag="zT")
                    nc.tensor.matmul(out=zT[:], lhsT=c_We, rhs=edg[:, csl],
                                     start=True, stop=False)
                    nc.tensor.matmul(out=zT[:], lhsT=c_Ws, rhs=srt[:, csl],
                                     start=False, stop=False)
                    nc.tensor.matmul(out=zT[:], lhsT=hr_t, rhs=ohne[:],
                                     start=False, stop=True)

                    x = sb.tile([P, W], F32R, tag="x")
                    nc.scalar.activation(x[:], zT[:], PRELU, alpha=0.01)

                    lg = ps1.tile([8, W], F32, tag="aux")
                    nc.tensor.matmul(out=lg[:], lhsT=c_ablk, rhs=x[:],
                                     start=True, stop=True)
                    exT = sb.tile([8, W], F32R, tag="exT")
                    nc.scalar.activation(exT[:], lg[:], EXP)

                    exP = ps1.tile([P, ncg * 8], F32, tag="aux")
                    for c in range(ncg):
                        nc.tensor.matmul(
                            out=exP[:, c * 8:(c + 1) * 8],
                            lhsT=exT[:, c * P:(c + 1) * P], rhs=c_id8,
                            start=True, stop=True,
                        )

                    spj = ps_spj.tile([P, ncg, 256], F32, tag="spj")
                    for c in range(ncg):
                        nc.tensor.matmul(
                            out=spj[:, c, :],
                            lhsT=srt[:, (gc0 + c) * P:(gc0 + c + 1) * P],
                            rhs=c_WsPad, start=True, stop=True,
                        )

                    rhs4 = sb.tile([P, ncg * 256], F32R, tag="rhs4")
                    r4v = rhs4[:].rearrange("p (c w) -> p c w", c=ncg)
                    # fill cols 128:256 of each chunk block with 16 replicas of
                    # ex -- col 128:136 is the real denominator input, the rest
                    # keeps the 256-wide scatter rhs fully initialized
                    _exv = exP[:, 0:ncg * 8].rearrange("p (c h) -> p c h", c=ncg)
                    _exb = bass.AP(_exv.tensor, _exv.offset,
                                   [_exv.ap[0], _exv.ap[1], [0, 16], _exv.ap[2]])
                    nc.scalar.activation(
                        r4v[:, :, 128:256].rearrange("p c (r h) -> p c r h", r=16),
                        _exb,
                        COPY,
                    )
                    nc.vector.tensor_tensor(
                        out=r4v[:, :, 0:128],
                        in0=spj[:, :, 0:128],
                        in1=r4v[:, :, 128:136].to_broadcast([P, ncg, 8, 16]),
                        op=MUL,
                    )

                    for c in range(ncg):
                        ohen = sb.tile([P, P], F32R, tag="ohen")
                        nc.vector.tensor_scalar(
                            out=ohen[:], in0=c_iota,
                            scalar1=rloc[:, t * T_max + gc0 + c: t * T_max + gc0 + c + 1],
                            scalar2=None, op0=EQ,
                        )
                        n_sc += 1
                        nc.tensor.matmul(
                            out=acc[:],
                            lhsT=ohen[:],
                            rhs=rhs4[:, c * 256:(c + 1) * 256],
                            start=(n_sc == 1), stop=(n_sc == T_max),
                        )

                # ---- epilogue ----
                dsb = sb.tile([P, 8], F32, tag="dsb")
                nc.vector.tensor_scalar(out=dsb[:], in0=acc[:, 128:136],
                                        scalar1=1e-30, scalar2=None, op0=ADD)
                rec = sb.tile([P, 8], F32, tag="rec")
                nc.vector.reciprocal(out=rec[:], in_=dsb[:])
                ot = sb.tile([P, P], F32, tag="ot")
                nc.vector.tensor_tensor(
                    out=ot[:].rearrange("p (h j) -> p h j", h=8),
                    in0=acc[:, 0:128].rearrange("p (h j) -> p h j", h=8),
                    in1=rec[:].to_broadcast([P, 8, 16]),
                    op=MUL,
                )
                nc.sync.dma_start(out=d_out[t * P:(t + 1) * P, :], in_=ot[:])

    nc.compile()

    in_maps = [
        dict(EDG=EDG[ci], SRT=SRT[ci], RROW=RROW[ci], RLOC=RLOC[ci],
             NLT=NLT[ci], CONST=CONST, ZER=np.zeros((P, 1024), np.float32))
        for ci in range(NCORES)
    ]
    bench_iters = int(os.environ.get("GAT_BENCH", "0"))
    results = _run_pjrt(nc, in_maps, NCORES, bench_iters)
    out = np.concatenate(
        [results[ci]["OUT"][:NLOC] for ci in range(NCORES)], axis=0
    )
    return out.astype(np.float32)


def _run_pjrt(nc, in_maps, n_cores, bench_iters=0):
    """Execute the compiled module on the PJRT/axon devices; optionally
    re-run with pre-staged device inputs to measure steady-state latency."""
    global LAST_EXEC_NS, LAST_BENCH_NS
    import time as _time
    import jax
    from jax.sharding import Mesh, PartitionSpec, NamedSharding
    from jax.experimental.shard_map import shard_map
    import concourse.mybir as _mb
    from concourse import bass2jax as _b2j

    _b2j.install_neuronx_cc_hook()

    in_names, out_names, out_avals, zero_outs = [], [], [], []
    for alloc in nc.m.functions[0].allocations:
        if not isinstance(_mb.MemoryLocationSet, type) or not isinstance(alloc, _mb.MemoryLocationSet):
            continue
        name = alloc.memorylocations[0].name
        if alloc.kind == "ExternalInput":
            if nc.partition_id_tensor is None or name != nc.partition_id_tensor.name:
                in_names.append(name)
        elif alloc.kind == "ExternalOutput":
            out_names.append(name)
            shape = tuple(alloc.tensor_shape)
            dtype = _mb.dt.np(alloc.dtype)
            out_avals.append(jax.core.ShapedArray(shape, dtype))
            zero_outs.append(np.zeros(shape, dtype))
    n_params = len(in_names)
    n_outs = len(out_avals)
    in_names = in_names + out_names

    part_name = nc.partition_id_tensor.name if nc.partition_id_tensor else None
    if part_name is not None:
        in_names.append(part_name)

    def _body(*args):
        operands = list(args)
        if part_name is not None:
            operands.append(_b2j.partition_id_tensor())
        outs = _b2j._bass_exec_p.bind(
            *operands,
            out_avals=tuple(out_avals),
            in_names=tuple(in_names),
            out_names=tuple(out_names),
            lowering_input_output_aliases=(),
            sim_require_finite=True,
            sim_require_nnan=True,
            nc=nc,
        )
        return tuple(outs)

    devices = jax.devices()[:n_cores]
    mesh = Mesh(np.asarray(devices), ("core",))
    in_specs = (PartitionSpec("core"),) * (n_params + n_outs)
    out_specs = (PartitionSpec("core"),) * n_outs
    fn = jax.jit(
        shard_map(_body, mesh=mesh, in_specs=in_specs,
                  out_specs=out_specs, check_rep=False),
        keep_unused=True,
    )
    sh = NamedSharding(mesh, PartitionSpec("core"))
    concat_in = [
        jax.device_put(
            np.concatenate([np.asarray(in_maps[c][in_names[i]])
                            for c in range(n_cores)], axis=0), sh)
        for i in range(n_params)
    ]
    concat_zeros = [
        jax.device_put(np.zeros((n_cores * z.shape[0], *z.shape[1:]), z.dtype), sh)
        for z in zero_outs
    ]
    out_arrs = fn(*concat_in, *concat_zeros)
    jax.block_until_ready(out_arrs)

    if bench_iters > 0:
        times = []
        for _ in range(bench_iters):
            t0 = _time.perf_counter()
            o = fn(*concat_in, *concat_zeros)
            jax.block_until_ready(o)
            times.append(_time.perf_counter() - t0)
        LAST_BENCH_NS = int(min(times) * 1e9)
        LAST_EXEC_NS = LAST_BENCH_NS

    np_outs = [np.asarray(a) for a in out_arrs]
    return [
        {name: np_outs[i].reshape(n_cores, *out_avals[i].shape)[c]
         for i, name in enumerate(out_names)}
        for c in range(n_cores)
    ]

